# revision 1
# baseline (speedup 1.0000x reference)
"""DictionaryLearningOMP forward on 8 TRN2 NeuronCores.

Reference computes out = (pinv(D) @ X).T with D = dictionary.T [256,512],
X = z_e [256,65536].  Equivalently out = X.T @ pinv(dictionary), where
pinv(dictionary) is [256,512].

Sharding: data-parallel along the N=65536 column dim -> 8 shards of 8192
columns.  The small [256,512] pinverse is computed once on host (f64) and
replicated to every core.  Each core computes out_shard[8192,512] =
x_shard.T @ dpt on the PE array (contract dim 256 = 2x128 chunks,
PSUM tiles [128,512]) and writes its slice; host concatenates.

Precision modes (KERNEL_MODE env; shipped default below):
  f16     in f16 / f16 matmul / out f16 (host upcasts)   ~12 MB DMA per core
  f32r    in f32 / float32r matmul / out f32             ~24.5 MB per core
  f32     in f32 / float32 matmul / out f32 (4x PE cost)
  bf16x3  in bf16 hi+lo / 3-way split matmul / out f32   (~fp32 accuracy)
"""

import os

import numpy as np

import concourse.bacc as bacc
import concourse.bass as bass
import concourse.mybir as mybir
import concourse.tile as tile
from concourse.bass_utils import run_bass_kernel_spmd

DIM = 256  # contraction dim (data dimension)
KATOMS = 512  # codebook size (output cols)
NTOT = 65536  # total signal columns
NCORES = 8
NSHARD = NTOT // NCORES  # 8192 columns per core

MODE = os.environ.get("KERNEL_MODE", "f16")

LAST_RESULT = None  # BassKernelResults of the most recent run (for test.py)

_cache = {}


def _mode_cfg(mode):
    dt = mybir.dt
    if mode == "f16":
        # in f16, out f16; 1MB loads ([128,2,2048] f16), 512KB stores (G=4)
        return dict(in_dt=dt.float16, out_dt=dt.float16, nterms=1, nbig=2048, g=4)
    if mode == "f32r":
        return dict(in_dt=dt.float32r, out_dt=dt.float32, nterms=1, nbig=1024, g=4)
    if mode == "f32":
        return dict(in_dt=dt.float32, out_dt=dt.float32, nterms=1, nbig=1024, g=4)
    if mode == "bf16x3":
        return dict(in_dt=dt.bfloat16, out_dt=dt.float32, nterms=3, nbig=2048, g=4)
    raise ValueError(mode)


def _build_module(mode):
    cfg = _mode_cfg(mode)
    in_dt, out_dt = cfg["in_dt"], cfg["out_dt"]
    NBIG, G = cfg["nbig"], cfg["g"]
    f32 = mybir.dt.float32
    nterms = cfg["nterms"]
    # term list: for split modes, (x_idx, d_idx) operand pairs to accumulate
    terms = [(0, 0)] if nterms == 1 else [(0, 0), (1, 0), (0, 1)]
    nxa = 2 if nterms > 1 else 1  # number of x input arrays (hi/lo)
    nda = 2 if nterms > 1 else 1

    nc = bacc.Bacc("TRN2", target_bir_lowering=False, debug=False)

    xs = [
        nc.dram_tensor(f"x{i}", [DIM, NSHARD], in_dt, kind="ExternalInput")
        for i in range(nxa)
    ]
    dps = [
        nc.dram_tensor(f"dpt{i}", [DIM, KATOMS], in_dt, kind="ExternalInput")
        for i in range(nda)
    ]
    out = nc.dram_tensor("out", [NSHARD, KATOMS], out_dt, kind="ExternalOutput")

    # fold the two 128-row contraction chunks into the partition dim
    xs_v = [x.rearrange("(j p) n -> p j n", p=128) for x in xs]
    out_v = out.rearrange("(m g p) k -> m p g k", p=128, g=G)

    n_sub = NBIG // 128  # psum tiles per x load
    with tile.TileContext(nc) as tc:
        with (
            tc.tile_pool(name="dict", bufs=1) as dict_pool,
            tc.tile_pool(name="xin", bufs=4) as xin_pool,
            tc.tile_pool(name="outs", bufs=4) as out_pool,
            tc.tile_pool(name="psum", bufs=7, space=bass.MemorySpace.PSUM) as psum_pool,
            tc.tile_pool(name="wups", bufs=1, space=bass.MemorySpace.PSUM) as wu_pool,
        ):
            # PE warm-up: ~4us of dummy matmuls so HAM un-throttles the PE
            # clock (1.2 -> 2.4 GHz) while the first loads are in flight.
            # iota fills run on GpSimd, which is idle from ~3.4us.
            wu_lhs = dict_pool.tile([128, 128], in_dt, tag="wu_lhs")
            wu_rhs = dict_pool.tile([128, KATOMS], in_dt, tag="wu_rhs")
            nc.gpsimd.iota(
                wu_lhs[:], [[1, 128]], channel_multiplier=0,
                allow_small_or_imprecise_dtypes=True,
            )
            nc.gpsimd.iota(
                wu_rhs[:], [[1, KATOMS]], channel_multiplier=0,
                allow_small_or_imprecise_dtypes=True,
            )
            wu_ps = wu_pool.tile([128, KATOMS], f32, tag="wu_ps")
            NWU = 8
            for w in range(NWU):
                nc.tensor.matmul(
                    wu_ps[:], wu_lhs[:], wu_rhs[:],
                    start=(w == 0), stop=(w == NWU - 1),
                )

            dpt_sbs = []
            for i, dp in enumerate(dps):
                dpt_sb = dict_pool.tile([128, 2, KATOMS], in_dt, tag=f"dict{i}")
                nc.gpsimd.dma_start(dpt_sb[:], dp.rearrange("(j p) k -> p j k", p=128))
                dpt_sbs.append(dpt_sb)

            # split the first load chunk small so the PE/copy/store pipeline
            # primes as early as possible
            chunks = []
            pos = 0
            for w in [512, NBIG - 512] + [NBIG] * (NSHARD // NBIG - 1):
                chunks.append((pos, w))
                pos += w

            gi = 0  # index within current output group
            ot = None
            tiles_done = 0
            for ci, (n0, w) in enumerate(chunks):
                xts = []
                for i, xv in enumerate(xs_v):
                    xt = xin_pool.tile([128, 2, w], in_dt, tag=f"x{i}")
                    nc.gpsimd.dma_start(xt[:], xv[:, :, n0 : n0 + w])
                    xts.append(xt)
                for s in range(w // 128):
                    ps = psum_pool.tile([128, KATOMS], f32)
                    nmm = len(terms) * 2
                    mi = 0
                    for xi, di in terms:
                        for j in range(2):
                            nc.tensor.matmul(
                                ps[:],
                                xts[xi][:, j, s * 128 : (s + 1) * 128],
                                dpt_sbs[di][:, j, :],
                                start=(mi == 0),
                                stop=(mi == nmm - 1),
                            )
                            mi += 1
                    if gi == 0:
                        ot = out_pool.tile([128, G, KATOMS], out_dt, tag="ot")
                    # split psum->sbuf copies evenly between DVE and ACT
                    if (gi % 2) == 0:
                        nc.vector.tensor_copy(ot[:, gi, :], ps[:])
                    else:
                        nc.scalar.copy(ot[:, gi, :], ps[:])
                    gi += 1
                    tiles_done += 1
                    if gi == G:
                        m = tiles_done // G - 1
                        nc.sync.dma_start(out_v[m], ot[:])
                        gi = 0

    nc.compile()
    return nc


def _get_module(mode):
    if mode not in _cache:
        _cache[mode] = _build_module(mode)
    return _cache[mode]


def _split_hi_lo(a, dtype):
    hi = a.astype(dtype)
    lo = (a - hi.astype(np.float32)).astype(dtype)
    return hi, lo


def kernel(z_e, dictionary):
    import ml_dtypes

    z_e = np.asarray(z_e, dtype=np.float32)
    dictionary = np.asarray(dictionary, dtype=np.float32)
    assert z_e.shape == (DIM, NTOT), z_e.shape
    assert dictionary.shape == (KATOMS, DIM), dictionary.shape

    # pinv(D).T = pinv(D.T) = pinv(dictionary): [256, 512].  Tiny; computed
    # in f64 on host once, replicated to all cores.
    dpt = np.linalg.pinv(dictionary.astype(np.float64)).astype(np.float32)

    mode = MODE
    nc = _get_module(mode)

    if mode == "f16":
        xs = [z_e.astype(np.float16)]
        dps = [dpt.astype(np.float16)]
    elif mode == "bf16x3":
        xh, xl = _split_hi_lo(z_e, ml_dtypes.bfloat16)
        dh, dl = _split_hi_lo(dpt, ml_dtypes.bfloat16)
        xs = [xh, xl]
        dps = [dh, dl]
    else:
        xs = [z_e]
        dps = [np.ascontiguousarray(dpt)]

    in_maps = []
    for i in range(NCORES):
        m = {}
        for k, x in enumerate(xs):
            m[f"x{k}"] = np.ascontiguousarray(x[:, i * NSHARD : (i + 1) * NSHARD])
        for k, dp in enumerate(dps):
            m[f"dpt{k}"] = np.ascontiguousarray(dp)
        in_maps.append(m)

    res = run_bass_kernel_spmd(nc, in_maps, core_ids=list(range(NCORES)))
    global LAST_RESULT
    LAST_RESULT = res
    outs = [r["out"] for r in res.results]
    full = np.concatenate(outs, axis=0)
    if full.dtype != np.float32:
        full = full.astype(np.float32)
    return full



# revision 2
# speedup vs baseline: 1.1108x; 1.1108x over previous
"""DictionaryLearningOMP forward on 8 TRN2 NeuronCores.

Reference computes out = (pinv(D) @ X).T with D = dictionary.T [256,512],
X = z_e [256,65536].  Equivalently out = X.T @ pinv(dictionary), where
pinv(dictionary) is [256,512].

Sharding: data-parallel along the N=65536 column dim -> 8 shards of 8192
columns.  The small [256,512] pinverse is computed once on host (f64),
scaled by OUT_SCALE, cast to f16 and replicated to every core.

Per-core kernel (PE-bound design, ~27.3us matmul floor):
  - x shard [256,8192] f16 loaded in 6 chunks (triggers on Sync, issued
    first so DMA flows during the framework preamble tail).
  - out written TRANSPOSED as [512,8192] float8_e3m4 (4-bit mantissa;
    quantization rel-err ~1.3e-2 vs 2e-2 budget).  Transposed layout
    gives 2KB+ contiguous DMA lines per partition.  Host rescales,
    upcasts and transposes back.
  - matmul: lhsT = dict chunk [128d,128k] stationary, rhs = x window
    [128d,512n] moving, PSUM [128k,2x512] f32 (2 banks per tile).
  - PSUM->SBUF cast copies alternate Vector/Scalar engines.
  - PE warm-up via memset tiles (Vector) so HAM un-throttles while the
    first loads are in flight.
"""

import numpy as np

import concourse.bacc as bacc
import concourse.bass as bass
import concourse.mybir as mybir
import concourse.tile as tile
from concourse.bass_utils import run_bass_kernel_spmd

DIM = 256  # contraction dim (data dimension)
KATOMS = 512  # codebook size (output rows in transposed layout)
NTOT = 65536  # total signal columns
NCORES = 8
NSHARD = NTOT // NCORES  # 8192 columns per core

OUT_SCALE = 32.0  # folded into dict on host; out e3m4 holds out*32 (|v|<8.4)

# x load chunks (cols): small first chunks prime the PE pipeline early
X_CHUNKS = [(0, 512), (512, 512), (1024, 1024), (2048, 2048), (4096, 2048), (6144, 2048)]
# store groups (cols): big groups for 2KB DMA lines, small tail for fast drain
O_GROUPS = [(0, 2048), (2048, 2048), (4096, 2048), (6144, 1024), (7168, 512), (7680, 512)]
NWU = 5  # PE warm-up matmuls

LAST_RESULT = None  # BassKernelResults of the most recent run (for test.py)

_cache = {}


def _build_module():
    f32 = mybir.dt.float32
    in_dt = mybir.dt.float16
    out_dt = mybir.dt.float8e3  # e3m4

    nc = bacc.Bacc("TRN2", target_bir_lowering=False, debug=False)

    x = nc.dram_tensor("x0", [DIM, NSHARD], in_dt, kind="ExternalInput")
    dp = nc.dram_tensor("dpt0", [DIM, KATOMS], in_dt, kind="ExternalInput")
    out = nc.dram_tensor("out", [KATOMS, NSHARD], out_dt, kind="ExternalOutput")

    # fold the two 128-row contraction chunks into the partition dim
    xv = x.rearrange("(j p) n -> p j n", p=128)
    # transposed out: partition p holds dict-atom row c*128+p, cols contiguous
    out_v = out.rearrange("(c p) n -> p c n", p=128)

    def chunk_of(n0):
        for ci, (c0, w) in enumerate(X_CHUNKS):
            if c0 <= n0 < c0 + w:
                return ci, n0 - c0
        raise AssertionError(n0)

    with tile.TileContext(nc) as tc:
        with (
            tc.tile_pool(name="dict", bufs=1) as dict_pool,
            tc.tile_pool(name="xin", bufs=1) as xin_pool,
            tc.tile_pool(name="outs", bufs=1) as out_pool,
            tc.tile_pool(name="psum", bufs=3, space=bass.MemorySpace.PSUM) as psum_pool,
            tc.tile_pool(name="wups", bufs=1, space=bass.MemorySpace.PSUM) as wu_pool,
        ):
            # --- load triggers first: dict + all x chunks on Sync ---
            dpt_sb = dict_pool.tile([128, 2, KATOMS], in_dt, tag="dict")
            nc.sync.dma_start(dpt_sb[:], dp.rearrange("(j p) k -> p j k", p=128))
            xts = []
            for ci, (c0, w) in enumerate(X_CHUNKS):
                xt = xin_pool.tile([128, 2, w], in_dt, tag=f"x{ci}")
                nc.sync.dma_start(xt[:], xv[:, :, c0 : c0 + w])
                xts.append(xt)

            # --- PE warm-up: memset tiles (Vector), NWU dummy matmuls so HAM
            # un-throttles the PE clock while the first loads are in flight ---
            wu_lhs = dict_pool.tile([128, 128], in_dt, tag="wu_lhs")
            wu_rhs = dict_pool.tile([128, KATOMS], in_dt, tag="wu_rhs")
            nc.vector.memset(wu_lhs[:], 1.0)
            nc.vector.memset(wu_rhs[:], 1.0)
            wu_ps = wu_pool.tile([128, KATOMS], f32, tag="wu_ps")
            for w in range(NWU):
                nc.tensor.matmul(
                    wu_ps[:], wu_lhs[:], wu_rhs[:],
                    start=(w == 0), stop=(w == NWU - 1),
                )

            # --- main loop: 512-col windows, psum [128, 2, 512] (2 banks) ---
            cp_i = 0
            for gi, (g0, gw) in enumerate(O_GROUPS):
                ot = out_pool.tile([128, 4, gw], out_dt, tag=f"o{gi}")
                for wo in range(0, gw, 512):
                    ci, loc = chunk_of(g0 + wo)
                    xt = xts[ci]
                    for pi in range(2):  # k-chunk pairs (0,1) and (2,3)
                        ps = psum_pool.tile([128, 2, 512], f32)
                        for c2 in range(2):
                            c = pi * 2 + c2
                            for d in range(2):
                                nc.tensor.matmul(
                                    ps[:, c2, :],
                                    dpt_sb[:, d, c * 128 : (c + 1) * 128],
                                    xt[:, d, loc : loc + 512],
                                    start=(d == 0),
                                    stop=(d == 1),
                                )
                        dst = ot[:, pi * 2 : pi * 2 + 2, wo : wo + 512]
                        if (cp_i % 2) == 0:
                            nc.vector.tensor_copy(dst, ps[:])
                        else:
                            nc.scalar.copy(dst, ps[:])
                        cp_i += 1
                nc.gpsimd.dma_start(out_v[:, :, g0 : g0 + gw], ot[:])

    nc.compile()
    return nc


def _get_module():
    if "m" not in _cache:
        _cache["m"] = _build_module()
    return _cache["m"]


def kernel(z_e, dictionary):
    z_e = np.asarray(z_e, dtype=np.float32)
    dictionary = np.asarray(dictionary, dtype=np.float32)
    assert z_e.shape == (DIM, NTOT), z_e.shape
    assert dictionary.shape == (KATOMS, DIM), dictionary.shape

    # pinv(D).T = pinv(D.T) = pinv(dictionary): [256, 512].  Tiny; computed
    # in f64 on host once, scaled and replicated to all cores.
    dpt = np.linalg.pinv(dictionary.astype(np.float64)) * OUT_SCALE

    nc = _get_module()

    xf16 = z_e.astype(np.float16)
    dpf16 = np.ascontiguousarray(dpt.astype(np.float16))

    in_maps = []
    for i in range(NCORES):
        in_maps.append({
            "x0": np.ascontiguousarray(xf16[:, i * NSHARD : (i + 1) * NSHARD]),
            "dpt0": dpf16,
        })

    res = run_bass_kernel_spmd(nc, in_maps, core_ids=list(range(NCORES)))
    global LAST_RESULT
    LAST_RESULT = res
    outs = [r["out"].astype(np.float32) for r in res.results]  # [512, 8192] each
    full = np.concatenate(outs, axis=1) * (1.0 / OUT_SCALE)  # [512, 65536]
    return np.ascontiguousarray(full.T)


# revision 3
# speedup vs baseline: 1.1330x; 1.0200x over previous
"""DictionaryLearningOMP forward on 8 TRN2 NeuronCores.

Reference computes out = (pinv(D) @ X).T with D = dictionary.T [256,512],
X = z_e [256,65536].  Equivalently out = X.T @ pinv(dictionary), where
pinv(dictionary) is [256,512].

Sharding: data-parallel along the N=65536 column dim -> 8 shards of 8192
columns.  The small [256,512] pinverse is computed once on host (f64),
scaled by OUT_SCALE, cast to f16 and replicated to every core.

Per-core kernel (PE-bound design, ~27.6us matmul floor at f16):
  - x shard [256,8192] f16 loaded in 7 chunks, triggers on Sync HWDGE
    issued first; dict trigger on Scalar HWDGE (runs in parallel with
    Sync's x0 trigger) so both land ~8.5us.
  - out written TRANSPOSED as [512,8192] float8_e3m4 (4-bit mantissa;
    quantization rel-err ~1.33e-2 vs 2e-2 budget).  Host rescales,
    upcasts and transposes back.
  - matmul: lhsT = dict chunk [128d,128k] stationary, rhs = x window
    [128d,<=512n] moving, PSUM [128k,2,512] f32 (2 banks per tile).
  - PSUM->SBUF cast copies alternate Vector/Scalar engines.
  - PE warm-up (2 matmuls on Vector-memset tiles) rolls seamlessly into
    the real matmuls so the HAM p-state ramp is never reset.
  - tail: last out group is two 256-col windows so the final
    copy->trigger->store chain is short.
"""

import numpy as np

import concourse.bacc as bacc
import concourse.bass as bass
import concourse.mybir as mybir
import concourse.tile as tile
from concourse.bass_utils import run_bass_kernel_spmd

DIM = 256  # contraction dim (data dimension)
KATOMS = 512  # codebook size (output rows in transposed layout)
NTOT = 65536  # total signal columns
NCORES = 8
NSHARD = NTOT // NCORES  # 8192 columns per core

OUT_SCALE = 32.0  # folded into dict on host; out e3m4 holds out*32 (|v|<8.4)

# x load chunks (cols): tiny first chunks so the PE can start ~8.5us
X_CHUNKS = [(0, 256), (256, 256), (512, 512), (1024, 1024),
            (2048, 2048), (4096, 2048), (6144, 2048)]
# (group_start, group_width, [window widths])  -- windows <= 512 (psum bank)
# and aligned so each window lies inside one x chunk
O_GROUPS = [
    (0, 2048, [256, 256, 512, 512, 512]),
    (2048, 2048, [512, 512, 512, 512]),
    (4096, 2048, [512, 512, 512, 512]),
    (6144, 1024, [512, 512]),
    (7168, 512, [512]),
    (7680, 512, [256, 256]),  # short tail: small final copies + 256KB store
]
NWU = 2  # PE warm-up matmuls (512 rows each, ~427ns at mid p-state)

LAST_RESULT = None  # BassKernelResults of the most recent run (for test.py)

_cache = {}


def _build_module():
    f32 = mybir.dt.float32
    in_dt = mybir.dt.float16
    out_dt = mybir.dt.float8e3  # e3m4

    nc = bacc.Bacc("TRN2", target_bir_lowering=False, debug=False)

    x = nc.dram_tensor("x0", [DIM, NSHARD], in_dt, kind="ExternalInput")
    dp = nc.dram_tensor("dpt0", [DIM, KATOMS], in_dt, kind="ExternalInput")
    out = nc.dram_tensor("out", [KATOMS, NSHARD], out_dt, kind="ExternalOutput")

    # fold the two 128-row contraction chunks into the partition dim
    xv = x.rearrange("(j p) n -> p j n", p=128)
    # transposed out: partition p holds dict-atom row c*128+p, cols contiguous
    out_v = out.rearrange("(c p) n -> p c n", p=128)

    def chunk_of(n0):
        for ci, (c0, w) in enumerate(X_CHUNKS):
            if c0 <= n0 < c0 + w:
                return ci, n0 - c0
        raise AssertionError(n0)

    with tile.TileContext(nc) as tc:
        with (
            tc.tile_pool(name="dict", bufs=1) as dict_pool,
            tc.tile_pool(name="xin", bufs=1) as xin_pool,
            tc.tile_pool(name="outs", bufs=1) as out_pool,
            tc.tile_pool(name="psum", bufs=3, space=bass.MemorySpace.PSUM) as psum_pool,
            tc.tile_pool(name="wups", bufs=1, space=bass.MemorySpace.PSUM) as wu_pool,
        ):
            # --- load triggers first: x chunks on Sync HWDGE, dict on
            # Scalar HWDGE (parallel trigger paths -> both land ~8.5us) ---
            dpt_sb = dict_pool.tile([128, 2, KATOMS], in_dt, tag="dict")
            nc.scalar.dma_start(dpt_sb[:], dp.rearrange("(j p) k -> p j k", p=128))
            xts = []
            for ci, (c0, w) in enumerate(X_CHUNKS):
                xt = xin_pool.tile([128, 2, w], in_dt, tag=f"x{ci}")
                nc.sync.dma_start(xt[:], xv[:, :, c0 : c0 + w])
                xts.append(xt)

            # --- PE warm-up: memset tiles (Vector), NWU dummy matmuls so the
            # HAM p-state ramp starts while the first loads are in flight ---
            wu_lhs = dict_pool.tile([128, 128], in_dt, tag="wu_lhs")
            wu_rhs = dict_pool.tile([128, KATOMS], in_dt, tag="wu_rhs")
            nc.vector.memset(wu_lhs[:], 1.0)
            nc.vector.memset(wu_rhs[:], 1.0)
            wu_ps = wu_pool.tile([128, KATOMS], f32, tag="wu_ps")
            for w in range(NWU):
                nc.tensor.matmul(
                    wu_ps[:], wu_lhs[:], wu_rhs[:],
                    start=(w == 0), stop=(w == NWU - 1),
                )

            # --- main loop ---
            cp_i = 0
            for gi, (g0, gw, wins) in enumerate(O_GROUPS):
                ot = out_pool.tile([128, 4, gw], out_dt, tag=f"o{gi}")
                wo = 0
                for wsz in wins:
                    ci, loc = chunk_of(g0 + wo)
                    xt = xts[ci]
                    for pi in range(2):  # k-chunk pairs (0,1) and (2,3)
                        ps = psum_pool.tile([128, 2, 512], f32)
                        for c2 in range(2):
                            c = pi * 2 + c2
                            for d in range(2):
                                nc.tensor.matmul(
                                    ps[:, c2, :wsz],
                                    dpt_sb[:, d, c * 128 : (c + 1) * 128],
                                    xt[:, d, loc : loc + wsz],
                                    start=(d == 0),
                                    stop=(d == 1),
                                )
                        dst = ot[:, pi * 2 : pi * 2 + 2, wo : wo + wsz]
                        if (cp_i % 2) == 0:
                            nc.vector.tensor_copy(dst, ps[:, :, :wsz])
                        else:
                            nc.scalar.copy(dst, ps[:, :, :wsz])
                        cp_i += 1
                    wo += wsz
                nc.sync.dma_start(out_v[:, :, g0 : g0 + gw], ot[:])

    nc.compile()
    return nc


def _get_module():
    if "m" not in _cache:
        _cache["m"] = _build_module()
    return _cache["m"]


def kernel(z_e, dictionary):
    z_e = np.asarray(z_e, dtype=np.float32)
    dictionary = np.asarray(dictionary, dtype=np.float32)
    assert z_e.shape == (DIM, NTOT), z_e.shape
    assert dictionary.shape == (KATOMS, DIM), dictionary.shape

    # pinv(D).T = pinv(D.T) = pinv(dictionary): [256, 512].  Tiny; computed
    # in f64 on host once, scaled and replicated to all cores.
    dpt = np.linalg.pinv(dictionary.astype(np.float64)) * OUT_SCALE

    nc = _get_module()

    xf16 = z_e.astype(np.float16)
    dpf16 = np.ascontiguousarray(dpt.astype(np.float16))

    in_maps = []
    for i in range(NCORES):
        in_maps.append({
            "x0": np.ascontiguousarray(xf16[:, i * NSHARD : (i + 1) * NSHARD]),
            "dpt0": dpf16,
        })

    res = run_bass_kernel_spmd(nc, in_maps, core_ids=list(range(NCORES)))
    global LAST_RESULT
    LAST_RESULT = res
    outs = [r["out"].astype(np.float32) for r in res.results]  # [512, 8192] each
    full = np.concatenate(outs, axis=1) * (1.0 / OUT_SCALE)  # [512, 65536]
    return np.ascontiguousarray(full.T)
